# revision 67
# baseline (speedup 1.0000x reference)
"""BatchHardTripletMarginLoss on 8 Trainium2 NeuronCores.

Strategy (anchor-sharded, embeddings replicated):
  - Each of the 8 cores owns 512 anchor rows of the 4096x4096 distance matrix.
  - label mode (what setup_inputs() produces) redesign:
      * Host CLASS-SORTS the 4096 points and, per core, ROTATES the column
        order so the core's own-class columns start near column 0.  Every
        block's positive window then lives in a STATIC column slice.
      * Candidates cand = sq_j + S - 2*e_i.e_j (+BIG on same-class cols) are
        produced by 3 bf16 matmul passes per 512-col psum bank (two 128-row E
        passes + one aug pass: two ones-rows carrying a hi/lo split of
        sq_j + S, plus BIG*one-hot class rows); evicted to bf16 cand in
        1024-col pairs.  Preload streams on 3 DMA queues (SP/ACT/Pool) in
        quarter order q0,q2,q1,q3; PE is p-state-warmed by dummy matmuls.
      * Mining per 128-anchor block: hardest positive = bf16 max-reduce over
        the static window (positives carry +BIG so the window max is the row
        max); hardest negative = bf16 TensorTensor min-chain (2x mode) folded
        quarter-by-quarter as evictions land + final reduce; ONE DVE
        max_index pass over the full row recovers both first-occurrence
        indices (for the last block the hp find/gather runs early).
      * Indices stay in rotated coords; e[hp], e[hn] gathered via indirect
        DMA from a per-core pre-rotated bf16 table (u32 adds saturate on DVE,
        so no modular index arithmetic on device); d_pn^2 recomputed.
      * d_ap^2/d_an^2 come from the mined candidate values (+ per-anchor
        tail terms); d_pn reduces are batched off the critical tail.
  - Per-core output: [128, 8] per-block partials (nonzero-loss sums, counts);
    host reduces across cores/partitions/blocks.

Fallback device variants (arbitrary masks; unchanged from the baseline):
  disjoint: arbitrary masks with pos & neg disjoint: one combined bf16
            additive mask (+BIG pos / 0 neg / +MID neither).
  overlap:  fully general fallback (two additive masks, two match passes).
"""

import math
import os

import numpy as np
import ml_dtypes

N, D, NCORES = 4096, 256, 8
R = N // NCORES          # 512 anchors per core
P = 128                  # partitions
NBLK = R // P            # 4 anchor blocks per core
CW = 512                 # chunk width for fallback modes (psum free dim)
NCH = N // CW            # 8 chunks per row (fallback modes)
LCW = 512                # label-mode matmul chunk width (one psum bank)
LNCH = N // LCW          # 8 chunks per row (label mode)
LQW = 1024               # label-mode DMA quarter width
MARGIN = 0.2
MAXC = 126               # max classes for the label path (KA=2+C <= 128)
NEG_DVE = 1024           # label mode: neg-min cols mined on DVE (rest Pool)
DEBUG = False            # emit dbg/dbgi outputs (sim validation)

_CACHE = {}
_LAST_PREP_INFO = {}


def _build_label(mm_dtype_name: str, nclass: int, pad: int):
    """Class-sorted / per-core-rotated label-mode kernel."""
    import concourse.bass as bass
    import concourse.mybir as mybir
    import concourse.tile as tile
    from concourse import bacc

    f32 = mybir.dt.float32
    bf16 = mybir.dt.bfloat16
    u32 = mybir.dt.uint32
    mmdt = getattr(mybir.dt, mm_dtype_name)
    Alu = mybir.AluOpType
    Act = mybir.ActivationFunctionType

    KA = 2 + nclass  # aug contraction rows: sq_hi, sq_lo, C one-hot rows

    nc = bacc.Bacc("TRN2", target_bir_lowering=False, debug=False, num_devices=NCORES)

    # pesm [P, NBLK*3P]: per block (-2 E_blk)^T rows 0:128 | rows 128:256 | AUG
    #   (AUG rows: 0,1 = ones; 2..2+C = BIG*U_blk^T; rest 0)
    # etq  [P, 2N]: E^T rows 0:128 | rows 128:256, cols rotated per core
    # augr [P, N] rows 0:KA: sq_hi+S | sq_lo | U^T, cols rotated per core
    pesm_d = nc.dram_tensor("pesm", [P, NBLK * 3 * P], mmdt, kind="ExternalInput")
    etq_d = nc.dram_tensor("etq", [P, 2 * N], mmdt, kind="ExternalInput")
    augr_d = nc.dram_tensor("augr", [P, N], mmdt, kind="ExternalInput")
    e_full = nc.dram_tensor("e", [N, D], bf16, kind="ExternalInput")  # sorted+rotated
    # scal: packed [sqi_ap(4) | sqi_an(4) | thr(2)]
    scal_d = nc.dram_tensor("scal", [P, 10], f32, kind="ExternalInput")
    own_idx = nc.dram_tensor("own_idx", [P, NBLK], u32, kind="ExternalInput")
    out_d = nc.dram_tensor("out", [P, 2 * NBLK], f32, kind="ExternalOutput")
    if DEBUG:
        dbg_d = nc.dram_tensor("dbg", [P, NBLK * 4], f32, kind="ExternalOutput")
        dbgi_d = nc.dram_tensor("dbgi", [P, NBLK * 2], u32, kind="ExternalOutput")

    with tile.TileContext(nc) as tc:
        with (
            tc.tile_pool(name="consts", bufs=1) as consts,
            tc.tile_pool(name="cands", bufs=4) as candp,
            tc.tile_pool(name="psum", bufs=4, space="PSUM") as psump,
            tc.tile_pool(name="tree", bufs=2) as treep,
            tc.tile_pool(name="small", bufs=6) as smallp,
            tc.tile_pool(name="gath", bufs=4) as gathp,
            tc.tile_pool(name="tail", bufs=1) as tailp,
        ):
            et0q = [consts.tile([P, LQW], mmdt, tag=f"et0q{q}", name=f"et0q{q}") for q in range(4)]
            et1q = [consts.tile([P, LQW], mmdt, tag=f"et1q{q}", name=f"et1q{q}") for q in range(4)]
            augrq = [consts.tile([P, LQW], mmdt, tag=f"augrq{q}", name=f"augrq{q}") for q in range(4)]

            # Preload on three parallel DMA queues: SP (pesm + E_lo quarters),
            # ACT hwdge (E_hi quarters), Pool swdge (aug quarters + smalls).
            # Quarter order q0,q2,q1,q3 matches the chunk compute order so the
            # neg min-tree's first pair (chunks 0,1 vs 4,5) is ready early.
            pesm_all = consts.tile([P, NBLK * 3 * P], mmdt, tag="pesm_all")
            pesm_blks = [pesm_all[:, b * 3 * P : (b + 1) * 3 * P] for b in range(NBLK)]
            nc.sync.dma_start(out=pesm_all[:, 0 : 3 * P], in_=pesm_d[:, 0 : 3 * P])
            nc.scalar.dma_start(out=pesm_all[:, 3 * P :], in_=pesm_d[:, 3 * P :])
            for q in (0, 2, 1, 3):
                nc.sync.dma_start(out=et0q[q][:], in_=etq_d[:, q * LQW : (q + 1) * LQW])
                nc.scalar.dma_start(out=et1q[q][:], in_=etq_d[:, N + q * LQW : N + (q + 1) * LQW])
                nc.gpsimd.dma_start(out=augrq[q][0:KA, :], in_=augr_d[0:KA, q * LQW : (q + 1) * LQW])
            scal_sb = consts.tile([P, 10], f32, tag="scal")
            nc.gpsimd.dma_start(out=scal_sb[:], in_=scal_d[:])
            own_sb = consts.tile([P, NBLK], u32, tag="own")
            nc.gpsimd.dma_start(out=own_sb[:], in_=own_idx[:])

            # PE p-state warmup: keep the PE continuously busy from t~0 until
            # the first real matmuls so the ramp timer expires (it resets when
            # the PE goes idle)
            wmu = tailp.tile([P, 272], mmdt, tag="wmu")
            nc.vector.memset(wmu[:], 0.0)
            for _ in range(14):
                wps = psump.tile([P, LQW], f32, tag="ps")
                nc.tensor.matmul(
                    out=wps[0:16, 0:256], lhsT=wmu[:, 0:16], rhs=wmu[:, 16:272],
                    start=True, stop=True,
                )

            pmax_arr = tailp.tile([P, NBLK], f32, tag="pmax_arr")
            nmin_arr = tailp.tile([P, NBLK], f32, tag="nmin_arr")
            dpn2 = tailp.tile([P, NBLK], f32, tag="dpn2")
            scr_all = tailp.tile([P, NBLK * D], bf16, tag="scr_all")
            hpi = tailp.tile([P, NBLK], u32, tag="hpi")
            comb = tailp.tile([P, 8], bf16, tag="comb")
            nc.vector.memset(comb[:], -3.0e38)
            dbgi_sb = tailp.tile([P, NBLK * 2], u32, tag="dbgi_sb")
            dap2 = tailp.tile([P, NBLK], f32, tag="dap2")
            dan2 = tailp.tile([P, NBLK], f32, tag="dan2")
            dap = tailp.tile([P, NBLK], f32, tag="dap")
            dapM = tailp.tile([P, NBLK], f32, tag="dapM")
            vp = tailp.tile([P, NBLK], f32, tag="vp")
            vn = tailp.tile([P, NBLK], f32, tag="vn")
            vpu = tailp.tile([P, NBLK], u32, tag="vpu")
            valid = tailp.tile([P, NBLK], f32, tag="valid")
            out_sb = tailp.tile([P, 2 * NBLK], f32, tag="out_sb")

            for b in range(NBLK):
                # static positive window (rotated coords) for this block
                lo_b = max(0, b * P - pad + 1)
                hi_b = min(N, b * P + P + 2 * pad - 1)

                cand_b = candp.tile([P, N], bf16, tag="cand")

                lhs0 = pesm_blks[b][:, 0:P]
                lhs1 = pesm_blks[b][:, P : 2 * P]
                lhsa = pesm_blks[b][0:KA, 2 * P : 3 * P]
                HQ = N // 4
                tree = treep.tile([P, N // 4], bf16, tag="tree")
                last = b == NBLK - 1
                for q in (0, 2, 1, 3):
                    # one psum tile per 1024-col quarter; each 512-col half is
                    # its own accumulation group (matmul can't cross a bank)
                    ps = psump.tile([P, LQW], f32, tag="ps")
                    for h in (0, 1):
                        hs = slice(h * LCW, (h + 1) * LCW)
                        nc.tensor.matmul(out=ps[:, hs], lhsT=lhs0, rhs=et0q[q][:, hs], start=True, stop=False)
                        nc.tensor.matmul(out=ps[:, hs], lhsT=lhs1, rhs=et1q[q][:, hs], start=False, stop=False)
                        nc.tensor.matmul(out=ps[:, hs], lhsT=lhsa, rhs=augrq[q][0:KA, hs], start=False, stop=True)
                    nc.scalar.activation(cand_b[:, q * LQW : (q + 1) * LQW], ps[:], Act.Copy)
                    if q == (hi_b - 1) // LQW:
                        # positive mining: bf16 window max (positives carry
                        # +BIG so the window max is the row max); runs right
                        # after the last window chunk is evicted
                        win = cand_b[:, lo_b:hi_b].rearrange("p (t w) -> p t w", t=1)
                        nc.vector.tensor_reduce(
                            out=comb[:, 0:1], in_=win,
                            axis=mybir.AxisListType.X, op=Alu.max,
                        )
                        if last:
                            # tail shortening: find + gather hp early so only
                            # the hn gather chain trails the final max_index
                            # (scan [0:hi_b) so the result is in row coords)
                            idx8p = smallp.tile([P, 8], u32, tag="idx8p")
                            nc.vector.max_index(idx8p[:], comb[:], cand_b[:, 0:hi_b])
                            nc.vector.tensor_copy(hpi[:, b : b + 1], idx8p[:, 0:1])
                    if q == 2:
                        # negative min-tree chain: fold each quarter in as it
                        # is evicted; only one op + reduce trail the last one
                        nc.vector.tensor_tensor(
                            out=tree[:, 0:HQ], in0=cand_b[:, 0:HQ],
                            in1=cand_b[:, 2 * HQ : 3 * HQ], op=Alu.min,
                        )
                    if q == 1:
                        nc.vector.tensor_tensor(
                            out=tree[:, 0:HQ], in0=tree[:, 0:HQ],
                            in1=cand_b[:, HQ : 2 * HQ], op=Alu.min,
                        )
                    if q == 3:
                        nc.vector.tensor_tensor(
                            out=tree[:, 0:HQ], in0=tree[:, 0:HQ],
                            in1=cand_b[:, 3 * HQ : N], op=Alu.min,
                        )
                nc.scalar.activation(pmax_arr[:, b : b + 1], comb[:, 0:1], Act.Copy)
                if last:
                    # launch the hp gather ahead of the full-row find
                    hpw = smallp.tile([P, 1], u32, tag="hpw")
                    nc.vector.tensor_scalar_min(hpw[:], idx8p[:, 0:1], N - 1)
                    ehp = gathp.tile([P, D], bf16, tag="ehp")
                    nc.gpsimd.indirect_dma_start(
                        out=ehp[:], out_offset=None, in_=e_full[:],
                        in_offset=bass.IndirectOffsetOnAxis(ap=hpw[:], axis=0),
                    )

                # negative mining: reduce the folded tree
                nc.vector.tensor_reduce(
                    out=comb[:, 1:2], in_=tree[:, 0:HQ].rearrange("p (t w) -> p t w", t=1),
                    axis=mybir.AxisListType.X, op=Alu.min,
                )
                nc.scalar.activation(nmin_arr[:, b : b + 1], comb[:, 1:2], Act.Copy)
                if last:
                    # dpn-independent tail work, hoisted ahead of the final
                    # find so only a short chain trails the last gather
                    nc.vector.tensor_tensor(out=dap2[:], in0=pmax_arr[:], in1=scal_sb[:, 0:NBLK], op=Alu.add)
                    nc.vector.tensor_tensor(out=dan2[:], in0=nmin_arr[:], in1=scal_sb[:, NBLK : 2 * NBLK], op=Alu.add)
                    nc.vector.tensor_scalar_max(dap2[:], dap2[:], 0.0)
                    nc.vector.tensor_scalar_max(dan2[:], dan2[:], 0.0)
                    nc.scalar.activation(dap[:], dap2[:], Act.Sqrt)
                    nc.vector.tensor_scalar_add(dapM[:], dap[:], MARGIN)
                    nc.vector.tensor_tensor(out=vpu[:], in0=hpi[:], in1=own_sb[:], op=Alu.not_equal)
                    nc.vector.tensor_copy(vp[:], vpu[:])
                    nc.vector.tensor_scalar(
                        out=vn[:], in0=nmin_arr[:],
                        scalar1=scal_sb[:, 2 * NBLK + 1 : 2 * NBLK + 2],
                        scalar2=None, op0=Alu.is_le,
                    )
                    nc.vector.tensor_tensor(out=valid[:], in0=vp[:], in1=vn[:], op=Alu.mult)
                idx8 = smallp.tile([P, 8], u32, tag="idx8")
                nc.vector.max_index(idx8[:], comb[:], cand_b[:])
                if not last:
                    nc.vector.tensor_copy(hpi[:, b : b + 1], idx8[:, 0:1])

                # clamp (unmatched needles return -1 == u32 max); indices stay
                # in rotated coords -- the gather table is pre-rotated per core
                idxc = smallp.tile([P, 2], u32, tag="idxc")
                nc.vector.tensor_scalar_min(idxc[:], idx8[:, 0:2], N - 1)
                if DEBUG:
                    if last:
                        nc.vector.tensor_copy(dbgi_sb[:, b : b + 1], hpi[:, b : b + 1])
                    else:
                        nc.vector.tensor_copy(dbgi_sb[:, b : b + 1], idxc[:, 0:1])
                    nc.vector.tensor_copy(dbgi_sb[:, NBLK + b : NBLK + b + 1], idxc[:, 1:2])

                # gather e[hp], e[hn]; d_pn^2 = |e_hp - e_hn|^2
                if not last:
                    ehp = gathp.tile([P, D], bf16, tag="ehp")
                    nc.gpsimd.indirect_dma_start(
                        out=ehp[:], out_offset=None, in_=e_full[:],
                        in_offset=bass.IndirectOffsetOnAxis(ap=idxc[:, 0:1], axis=0),
                    )
                ehn = gathp.tile([P, D], bf16, tag="ehn")
                nc.gpsimd.indirect_dma_start(
                    out=ehn[:], out_offset=None, in_=e_full[:],
                    in_offset=bass.IndirectOffsetOnAxis(ap=idxc[:, 1:2], axis=0),
                )
                if last:
                    # batched d_pn^2 reduce for blocks 0..2: slotted between
                    # the last gather launch and its consumers so the in-order
                    # DVE stream fills the gather wait with useful work
                    nc.vector.tensor_reduce(
                        out=dpn2[:, 0 : NBLK - 1],
                        in_=scr_all[:, 0 : (NBLK - 1) * D].rearrange("p (t w) -> p t w", t=NBLK - 1),
                        axis=mybir.AxisListType.X, op=Alu.add,
                    )
                diff = gathp.tile([P, D], bf16, tag="diff")
                dpn_eng = nc.vector if b == NBLK - 1 else nc.gpsimd
                dpn_eng.tensor_tensor(
                    out=diff[:], in0=ehp[:], in1=ehn[:], op=Alu.subtract
                )
                dpn_eng.tensor_tensor(
                    out=scr_all[:, b * D : (b + 1) * D], in0=diff[:], in1=diff[:], op=Alu.mult
                )

            # ---- final tail: only the dpn-dependent chain remains ----
            nc.vector.tensor_reduce(
                out=dpn2[:, NBLK - 1 : NBLK],
                in_=scr_all[:, (NBLK - 1) * D : NBLK * D].rearrange("p (t w) -> p t w", t=1),
                axis=mybir.AxisListType.X, op=Alu.add,
            )
            dane2 = tailp.tile([P, NBLK], f32, tag="dane2")
            nc.vector.tensor_tensor(out=dane2[:], in0=dan2[:], in1=dpn2[:], op=Alu.min)
            dane = tailp.tile([P, NBLK], f32, tag="dane")
            nc.scalar.activation(dane[:], dane2[:], Act.Sqrt)
            s_t = tailp.tile([P, NBLK], f32, tag="s_t")
            nc.vector.tensor_tensor(out=s_t[:], in0=dapM[:], in1=dane[:], op=Alu.subtract)
            l_t = tailp.tile([P, NBLK], f32, tag="l_t")
            nc.vector.tensor_scalar_max(l_t[:], s_t[:], 0.0)
            gt = tailp.tile([P, NBLK], f32, tag="gt")
            nc.vector.tensor_scalar(
                out=gt[:], in0=s_t[:], scalar1=0.0, scalar2=None, op0=Alu.is_gt
            )
            nc.vector.tensor_tensor(out=out_sb[:, 0:NBLK], in0=l_t[:], in1=valid[:], op=Alu.mult)
            nc.vector.tensor_tensor(out=out_sb[:, NBLK : 2 * NBLK], in0=gt[:], in1=valid[:], op=Alu.mult)

            if DEBUG:
                dbg_sb = tailp.tile([P, NBLK * 4], f32, tag="dbg_sb")
                nc.scalar.activation(dbg_sb[:, 0:NBLK], pmax_arr[:], Act.Copy)
                nc.scalar.activation(dbg_sb[:, NBLK : 2 * NBLK], nmin_arr[:], Act.Copy)
                nc.scalar.activation(dbg_sb[:, 2 * NBLK : 3 * NBLK], dpn2[:], Act.Copy)
                nc.scalar.activation(dbg_sb[:, 3 * NBLK : 4 * NBLK], dap2[:], Act.Copy)
                nc.sync.dma_start(out=dbg_d[:], in_=dbg_sb[:])
                nc.sync.dma_start(out=dbgi_d[:], in_=dbgi_sb[:])

            nc.sync.dma_start(out=out_d[:], in_=out_sb[:])

    nc.finalize()
    return nc


def _build(mode: str, mm_dtype_name: str, nclass: int = 0, pad: int = 128):
    if mode == "label":
        return _build_label(mm_dtype_name, nclass, pad)

    import concourse.bass as bass
    import concourse.mybir as mybir
    import concourse.tile as tile
    from concourse import bacc

    f32 = mybir.dt.float32
    bf16 = mybir.dt.bfloat16
    u32 = mybir.dt.uint32
    mmdt = getattr(mybir.dt, mm_dtype_name)
    Alu = mybir.AluOpType
    Act = mybir.ActivationFunctionType

    disjoint = mode != "overlap"
    KA = 1  # contraction rows of the 3rd matmul

    nc = bacc.Bacc("TRN2", target_bir_lowering=False, debug=False, num_devices=NCORES)

    pesm_d = nc.dram_tensor("pesm", [P, 3 * R], mmdt, kind="ExternalInput")  # block-packed
    etq_d = nc.dram_tensor("etq", [P, 2 * N], mmdt, kind="ExternalInput")
    augr_d = nc.dram_tensor("augr", [P, N], mmdt, kind="ExternalInput")
    QW = N // 4  # 1024: quarter width
    e_full = nc.dram_tensor("e", [N, D], f32, kind="ExternalInput")
    sqi_ap = nc.dram_tensor("sqi_ap", [P, NBLK], f32, kind="ExternalInput")
    sqi_an = nc.dram_tensor("sqi_an", [P, NBLK], f32, kind="ExternalInput")
    thr = nc.dram_tensor("thr", [P, 2], f32, kind="ExternalInput")
    if disjoint:
        mc_d = nc.dram_tensor("mc", [R, N], bf16, kind="ExternalInput")
    else:
        mcp_d = nc.dram_tensor("mcp", [R, N], bf16, kind="ExternalInput")
        mcn_d = nc.dram_tensor("mcn", [R, N], bf16, kind="ExternalInput")
    out_d = nc.dram_tensor("out", [P, 2], f32, kind="ExternalOutput")
    if DEBUG:
        dbg_d = nc.dram_tensor("dbg", [P, NBLK * 4], f32, kind="ExternalOutput")
        dbgi_d = nc.dram_tensor("dbgi", [P, NBLK * 2], mybir.dt.uint32, kind="ExternalOutput")

    with tile.TileContext(nc) as tc:
        with (
            tc.tile_pool(name="consts", bufs=1) as consts,
            tc.tile_pool(name="masks", bufs=2) as maskp,
            tc.tile_pool(name="cands", bufs=4) as candp,
            tc.tile_pool(name="psum", bufs=4, space="PSUM") as psump,
            tc.tile_pool(name="small", bufs=6) as smallp,
            tc.tile_pool(name="gath", bufs=4) as gathp,
            tc.tile_pool(name="tail", bufs=1) as tailp,
        ):
            pesm_blks = [consts.tile([P, 3 * P], mmdt, tag=f"pesm{b}", name=f"pesm{b}") for b in range(NBLK)]
            et0q = [consts.tile([P, QW], mmdt, tag=f"et0q{q}", name=f"et0q{q}") for q in range(4)]
            et1q = [consts.tile([P, QW], mmdt, tag=f"et1q{q}", name=f"et1q{q}") for q in range(4)]
            augrq = [consts.tile([P, QW], mmdt, tag=f"augrq{q}", name=f"augrq{q}") for q in range(4)]

            def _dma_q(q):
                nc.sync.dma_start(out=et0q[q][:], in_=etq_d[:, q * QW : (q + 1) * QW])
                nc.sync.dma_start(out=et1q[q][:], in_=etq_d[:, N + q * QW : N + (q + 1) * QW])
                nc.sync.dma_start(out=augrq[q][0:KA, :], in_=augr_d[0:KA, q * QW : (q + 1) * QW])

            nc.sync.dma_start(out=pesm_blks[0][:], in_=pesm_d[:, 0 : 3 * P])
            _dma_q(0)
            for b in range(1, NBLK):
                nc.sync.dma_start(out=pesm_blks[b][:], in_=pesm_d[:, b * 3 * P : (b + 1) * 3 * P])
            for q in range(1, 4):
                _dma_q(q)
            sqi_ap_sb = consts.tile([P, NBLK], f32, tag="sqi_ap")
            nc.sync.dma_start(out=sqi_ap_sb[:], in_=sqi_ap[:])
            sqi_an_sb = consts.tile([P, NBLK], f32, tag="sqi_an")
            nc.sync.dma_start(out=sqi_an_sb[:], in_=sqi_an[:])
            thr_sb = consts.tile([P, 2], f32, tag="thr")
            nc.sync.dma_start(out=thr_sb[:], in_=thr[:])

            pmax_arr = tailp.tile([P, NBLK], f32, tag="pmax_arr")
            nmin_arr = tailp.tile([P, NBLK], f32, tag="nmin_arr")
            dpn2 = tailp.tile([P, NBLK], f32, tag="dpn2")
            dbgi_sb = tailp.tile([P, NBLK * 2], u32, tag="dbgi_sb")

            for b in range(NBLK):
                rows = slice(b * P, (b + 1) * P)
                if disjoint:
                    mc_b = maskp.tile([P, N], bf16, tag="mc")
                    nc.sync.dma_start(out=mc_b[:], in_=mc_d[rows, :])
                else:
                    mcp_b = maskp.tile([P, N], bf16, tag="mcp")
                    nc.sync.dma_start(out=mcp_b[:], in_=mcp_d[rows, :])
                    mcn_b = maskp.tile([P, N], bf16, tag="mcn")
                    nc.sync.dma_start(out=mcn_b[:], in_=mcn_d[rows, :])

                cand_b = candp.tile([P, N], f32, tag="cand")
                if not disjoint:
                    ncand_b = candp.tile([P, N], f32, tag="ncand")
                pm = smallp.tile([P, NCH], f32, tag="pm")
                nm = smallp.tile([P, NCH], f32, tag="nm")

                lhs0 = pesm_blks[b][:, 0:P]
                lhs1 = pesm_blks[b][:, P : 2 * P]
                lhsa = pesm_blks[b][0:KA, 2 * P : 3 * P]
                for c in range(NCH):
                    cs = slice(c * CW, (c + 1) * CW)
                    q, qs = c // 2, slice((c % 2) * CW, (c % 2) * CW + CW)
                    ps = psump.tile([P, CW], f32, tag="ps")
                    nc.tensor.matmul(out=ps[:], lhsT=lhs0, rhs=et0q[q][:, qs], start=True, stop=False)
                    nc.tensor.matmul(out=ps[:], lhsT=lhs1, rhs=et1q[q][:, qs], start=False, stop=False)
                    nc.tensor.matmul(out=ps[:], lhsT=lhsa, rhs=augrq[q][0:KA, qs], start=False, stop=True)
                    if disjoint:
                        nc.vector.tensor_tensor(
                            out=cand_b[:, cs], in0=ps[:], in1=mc_b[:, cs], op=Alu.add
                        )
                        nc.vector.tensor_reduce(
                            out=pm[:, c : c + 1], in_=cand_b[:, cs],
                            axis=mybir.AxisListType.X, op=Alu.max,
                        )
                        nc.vector.tensor_reduce(
                            out=nm[:, c : c + 1], in_=cand_b[:, cs],
                            axis=mybir.AxisListType.X, op=Alu.min,
                        )
                    else:
                        nc.vector.tensor_tensor(
                            out=cand_b[:, cs], in0=ps[:], in1=mcp_b[:, cs], op=Alu.add
                        )
                        nc.vector.tensor_reduce(
                            out=pm[:, c : c + 1], in_=cand_b[:, cs],
                            axis=mybir.AxisListType.X, op=Alu.max,
                        )
                        nc.vector.tensor_tensor(
                            out=ncand_b[:, cs], in0=ps[:], in1=mcn_b[:, cs], op=Alu.add
                        )
                        nc.vector.tensor_reduce(
                            out=nm[:, c : c + 1], in_=ncand_b[:, cs],
                            axis=mybir.AxisListType.X, op=Alu.min,
                        )

                psort = smallp.tile([P, 8], f32, tag="psort")
                nsort = smallp.tile([P, 8], f32, tag="nsort")
                nc.vector.max(psort[:], pm[:])
                nc.vector.max(nsort[:], nm[:])
                nc.scalar.activation(pmax_arr[:, b : b + 1], psort[:, 0:1], Act.Copy)
                nc.scalar.activation(nmin_arr[:, b : b + 1], nsort[:, 7:8], Act.Copy)

                if disjoint:
                    comb = smallp.tile([P, 8], f32, tag="comb")
                    nc.vector.tensor_copy(comb[:], psort[:])
                    nc.vector.tensor_copy(comb[:, 1:2], nsort[:, 7:8])
                    idx8 = smallp.tile([P, 8], u32, tag="idx8")
                    nc.vector.max_index(idx8[:], comb[:], cand_b[:])
                    hp_idx = idx8[:, 0:1]
                    hn_idx = idx8[:, 1:2]
                else:
                    pidx8 = smallp.tile([P, 8], u32, tag="pidx8")
                    nc.vector.max_index(pidx8[:], psort[:], cand_b[:])
                    nidx8 = smallp.tile([P, 8], u32, tag="nidx8")
                    nc.vector.max_index(nidx8[:], nsort[:], ncand_b[:])
                    hp_idx = pidx8[:, 0:1]
                    hn_idx = nidx8[:, 7:8]

                # clamp indices (unmatched needles return -1 == u32 max)
                idxc = smallp.tile([P, 2], u32, tag="idxc")
                nc.vector.tensor_scalar_min(idxc[:, 0:1], hp_idx, N - 1)
                nc.vector.tensor_scalar_min(idxc[:, 1:2], hn_idx, N - 1)
                if DEBUG:
                    nc.vector.tensor_copy(dbgi_sb[:, b : b + 1], idxc[:, 0:1])
                    nc.vector.tensor_copy(dbgi_sb[:, NBLK + b : NBLK + b + 1], idxc[:, 1:2])
                # gather e[hp], e[hn]; d_pn^2 = |e_hp - e_hn|^2
                ehp = gathp.tile([P, D], f32, tag="ehp")
                ehn = gathp.tile([P, D], f32, tag="ehn")
                nc.gpsimd.indirect_dma_start(
                    out=ehp[:], out_offset=None, in_=e_full[:],
                    in_offset=bass.IndirectOffsetOnAxis(ap=idxc[:, 0:1], axis=0),
                )
                nc.gpsimd.indirect_dma_start(
                    out=ehn[:], out_offset=None, in_=e_full[:],
                    in_offset=bass.IndirectOffsetOnAxis(ap=idxc[:, 1:2], axis=0),
                )
                diff = gathp.tile([P, D], f32, tag="diff")
                scr = gathp.tile([P, D], f32, tag="scr")
                nc.gpsimd.tensor_tensor(
                    out=diff[:], in0=ehp[:], in1=ehn[:], op=Alu.subtract
                )
                nc.gpsimd.tensor_tensor(
                    out=scr[:], in0=diff[:], in1=diff[:], op=Alu.mult
                )
                nc.vector.tensor_reduce(
                    out=dpn2[:, b : b + 1], in_=scr[:],
                    axis=mybir.AxisListType.X, op=Alu.add,
                )

            # ---- tail: per-anchor losses ([P, NBLK] arrays) ----
            dap2 = tailp.tile([P, NBLK], f32, tag="dap2")
            dan2 = tailp.tile([P, NBLK], f32, tag="dan2")
            nc.vector.tensor_tensor(out=dap2[:], in0=pmax_arr[:], in1=sqi_ap_sb[:], op=Alu.add)
            nc.vector.tensor_tensor(out=dan2[:], in0=nmin_arr[:], in1=sqi_an_sb[:], op=Alu.add)
            nc.vector.tensor_scalar_max(dap2[:], dap2[:], 0.0)
            nc.vector.tensor_scalar_max(dan2[:], dan2[:], 0.0)
            dane2 = tailp.tile([P, NBLK], f32, tag="dane2")
            nc.vector.tensor_tensor(out=dane2[:], in0=dan2[:], in1=dpn2[:], op=Alu.min)
            dap = tailp.tile([P, NBLK], f32, tag="dap")
            dane = tailp.tile([P, NBLK], f32, tag="dane")
            nc.scalar.activation(dap[:], dap2[:], Act.Sqrt)
            nc.scalar.activation(dane[:], dane2[:], Act.Sqrt)
            s_t = tailp.tile([P, NBLK], f32, tag="s_t")
            nc.vector.tensor_tensor(out=s_t[:], in0=dap[:], in1=dane[:], op=Alu.subtract)
            nc.vector.tensor_scalar_add(s_t[:], s_t[:], MARGIN)
            l_t = tailp.tile([P, NBLK], f32, tag="l_t")
            nc.vector.tensor_scalar_max(l_t[:], s_t[:], 0.0)
            gt = tailp.tile([P, NBLK], f32, tag="gt")
            nc.vector.tensor_scalar(
                out=gt[:], in0=s_t[:], scalar1=0.0, scalar2=None, op0=Alu.is_gt
            )
            vp = tailp.tile([P, NBLK], f32, tag="vp")
            vn = tailp.tile([P, NBLK], f32, tag="vn")
            nc.vector.tensor_scalar(
                out=vp[:], in0=pmax_arr[:], scalar1=thr_sb[:, 0:1], scalar2=None, op0=Alu.is_ge
            )
            nc.vector.tensor_scalar(
                out=vn[:], in0=nmin_arr[:], scalar1=thr_sb[:, 1:2], scalar2=None, op0=Alu.is_le
            )
            valid = tailp.tile([P, NBLK], f32, tag="valid")
            nc.vector.tensor_tensor(out=valid[:], in0=vp[:], in1=vn[:], op=Alu.mult)
            contrib = tailp.tile([P, NBLK], f32, tag="contrib")
            nc.vector.tensor_tensor(out=contrib[:], in0=l_t[:], in1=valid[:], op=Alu.mult)
            cntc = tailp.tile([P, NBLK], f32, tag="cntc")
            nc.vector.tensor_tensor(out=cntc[:], in0=gt[:], in1=valid[:], op=Alu.mult)

            if DEBUG:
                dbg_sb = tailp.tile([P, NBLK * 4], f32, tag="dbg_sb")
                nc.scalar.activation(dbg_sb[:, 0:NBLK], pmax_arr[:], Act.Copy)
                nc.scalar.activation(dbg_sb[:, NBLK : 2 * NBLK], nmin_arr[:], Act.Copy)
                nc.scalar.activation(dbg_sb[:, 2 * NBLK : 3 * NBLK], dpn2[:], Act.Copy)
                nc.scalar.activation(dbg_sb[:, 3 * NBLK : 4 * NBLK], dap2[:], Act.Copy)
                nc.sync.dma_start(out=dbg_d[:], in_=dbg_sb[:])
                nc.sync.dma_start(out=dbgi_d[:], in_=dbgi_sb[:])

            out_sb = tailp.tile([P, 2], f32, tag="out_sb")
            nc.vector.tensor_reduce(
                out=out_sb[:, 0:1], in_=contrib[:], axis=mybir.AxisListType.X, op=Alu.add
            )
            nc.vector.tensor_reduce(
                out=out_sb[:, 1:2], in_=cntc[:], axis=mybir.AxisListType.X, op=Alu.add
            )
            nc.sync.dma_start(out=out_d[:], in_=out_sb[:])

    nc.finalize()
    return nc


def _next_pow2(x: float) -> float:
    return float(2.0 ** math.ceil(math.log2(max(x, 1.0))))


def _detect_labels(pos: np.ndarray, neg: np.ndarray):
    """If (pos, neg) are label-derived (pos = same&~eye, neg = ~same) with
    <= MAXC classes, return int labels [N]; else None."""
    if pos.diagonal().any():
        return None
    same = pos.copy()
    np.fill_diagonal(same, True)
    if np.logical_xor(neg, ~same).any():
        return None
    lab = np.argmax(same, axis=1)  # first member of each row's class
    if not np.array_equal(same, lab[:, None] == lab[None, :]):
        return None
    if len(np.unique(lab)) > MAXC:
        return None
    return lab


def prep(embeddings, positives_mask, negatives_mask):
    """Host-side prep shared by kernel(), test.py and analyze.py.
    Returns (mode, nclass, in_maps); build params in _LAST_PREP_INFO."""
    emb = np.ascontiguousarray(embeddings, dtype=np.float32)
    pos = np.asarray(positives_mask).astype(bool)
    neg = np.asarray(negatives_mask).astype(bool)

    sq = (emb.astype(np.float64) ** 2).sum(axis=1).astype(np.float32)
    smax = float(sq.max())
    S = _next_pow2(smax)
    r_hi = 4.0 * smax + S
    MID = _next_pow2(r_hi * 1.1 + 4.0)

    lab = _detect_labels(pos, neg)
    bf = ml_dtypes.bfloat16
    if lab is not None:
        mode = "label"
        BIG = _next_pow2(2.0 * r_hi + 64.0)
        t_neg = (r_hi + BIG) / 2.0
        _, inv = np.unique(lab, return_inverse=True)
        nclass = int(inv.max()) + 1
        order = np.argsort(inv, kind="stable")
        inv_s = inv[order]
        e_s = np.ascontiguousarray(emb[order])
        sq_s = sq[order]
        sizes = np.bincount(inv_s, minlength=nclass)
        starts = np.zeros(nclass, np.int64)
        starts[1:] = np.cumsum(sizes)[:-1]
        pad = int(sizes.max())

        U_s = (inv_s[:, None] == np.arange(nclass)[None, :]).astype(np.float32)
        sqS = (sq_s + np.float32(S)).astype(np.float32)
        sqhi = sqS.astype(bf)
        sqlo = (sqS - sqhi.astype(np.float32)).astype(bf)
        # each core's 512 sorted anchors span only a contiguous handful of
        # classes; the aug matmul only needs one-hot rows for those
        span = [
            int(inv_s[(c + 1) * R - 1] - inv_s[c * R]) + 1 for c in range(NCORES)
        ]
        nc_eff = max(span)
        KA = 2 + nc_eff

        et = e_s.T.astype(bf)  # [D, N]
        thr = np.empty((P, 2), np.float32)
        thr[:, 0] = 0.0
        thr[:, 1] = t_neg

        in_maps = []
        for c in range(NCORES):
            rows = slice(c * R, (c + 1) * R)
            rot = int(starts[inv_s[c * R]])
            delta = c * R - rot

            et_rot = np.roll(et, -rot, axis=1)
            etq = np.concatenate([et_rot[0:P], et_rot[P : 2 * P]], axis=1)  # [P,2N]

            cls_lo = int(inv_s[c * R])
            ucore = U_s[:, cls_lo : cls_lo + span[c]]  # [N, span_c]
            augr = np.zeros((P, N), bf)
            augr[0] = np.roll(sqhi, -rot)
            augr[1] = np.roll(sqlo, -rot)
            augr[2 : 2 + span[c]] = np.roll(ucore.T, -rot, axis=1).astype(bf)

            pesm = np.zeros((P, NBLK * 3 * P), np.float32)
            etm2 = (-2.0 * e_s[rows]).T.astype(np.float32)  # [D, R]
            for b in range(NBLK):
                bs = slice(b * P, (b + 1) * P)
                o = b * 3 * P
                pesm[:, o : o + P] = etm2[0:P, bs]
                pesm[:, o + P : o + 2 * P] = etm2[P : 2 * P, bs]
                pesm[0, o + 2 * P : o + 3 * P] = 1.0
                pesm[1, o + 2 * P : o + 3 * P] = 1.0
                pesm[2 : 2 + span[c], o + 2 * P : o + 3 * P] = (
                    np.float32(BIG) * ucore[c * R + b * P : c * R + (b + 1) * P].T
                )

            sqi = sq_s[rows].reshape(NBLK, P).T.copy()  # [P, NBLK]
            own = (np.arange(R, dtype=np.uint32) + np.uint32(delta)).reshape(NBLK, P).T.copy()

            scal = np.empty((P, 10), np.float32)
            scal[:, 0:NBLK] = sqi - np.float32(S + BIG)
            scal[:, NBLK : 2 * NBLK] = sqi - np.float32(S)
            scal[:, 2 * NBLK : 2 * NBLK + 2] = thr
            in_maps.append({
                "pesm": pesm.astype(bf),
                "etq": etq,
                "augr": augr,
                "e": np.ascontiguousarray(np.roll(e_s, -rot, axis=0)).astype(bf),
                "scal": scal,
                "own_idx": own,
            })
        _LAST_PREP_INFO.clear()
        _LAST_PREP_INFO.update(
            mode=mode, nclass=nc_eff, pad=pad, order=order, rot_list=[
                int(starts[inv_s[c * R]]) for c in range(NCORES)
            ],
        )
        return mode, nc_eff, in_maps

    # ---- fallback modes (unchanged from baseline) ----
    nclass = 0
    BIG = 4.0 * MID
    if not bool(np.logical_and(pos, neg).any()):
        mode = "disjoint"
        t_pos, t_neg = 2.0 * MID, (MID + r_hi) / 2.0
    else:
        mode = "overlap"
        t_pos, t_neg = -BIG / 2.0, BIG / 2.0

    et = np.ascontiguousarray(emb.T)
    sqjs = (sq + np.float32(S)).astype(np.float32)
    if mode == "disjoint":
        mc_full = np.where(
            pos, np.float32(BIG), np.where(neg, np.float32(0.0), np.float32(MID))
        ).astype(bf)
    else:
        mcp_full = np.where(pos, np.float32(0.0), np.float32(-BIG)).astype(bf)
        mcn_full = np.where(neg, np.float32(0.0), np.float32(BIG)).astype(bf)

    thr = np.empty((P, 2), np.float32)
    thr[:, 0] = t_pos
    thr[:, 1] = t_neg

    etq = np.concatenate([et[0:P], et[P : 2 * P]], axis=1)  # [P, 2N], shared
    augr = np.zeros((P, N), np.float32)
    augr[0] = sqjs
    in_maps = []
    for c in range(NCORES):
        rows = slice(c * R, (c + 1) * R)
        sqi = sq[rows].reshape(NBLK, P).T.copy()  # [P, NBLK]
        pesm = np.zeros((P, 3 * R), np.float32)
        etm2 = (-2.0 * emb[rows]).T.astype(np.float32)
        for b in range(NBLK):
            bs = slice(b * P, (b + 1) * P)
            o = b * 3 * P
            pesm[:, o : o + P] = etm2[0:P, bs]
            pesm[:, o + P : o + 2 * P] = etm2[P : 2 * P, bs]
            pesm[0, o + 2 * P : o + 3 * P] = 1.0
        m = {
            "pesm": pesm,
            "etq": etq,
            "augr": augr,
            "e": emb,
            "sqi_an": sqi - np.float32(S),
            "thr": thr,
        }
        if mode == "disjoint":
            m["sqi_ap"] = sqi - np.float32(S + BIG)
            m["mc"] = np.ascontiguousarray(mc_full[rows])
        else:
            m["sqi_ap"] = sqi - np.float32(S)
            m["mcp"] = np.ascontiguousarray(mcp_full[rows])
            m["mcn"] = np.ascontiguousarray(mcn_full[rows])
        in_maps.append(m)
    _LAST_PREP_INFO.clear()
    _LAST_PREP_INFO.update(mode=mode, nclass=nclass, pad=128)
    return mode, nclass, in_maps


def get_nc(mode, nclass, pad):
    mm_default = "bfloat16" if mode == "label" else "float32r"
    mm_dtype = os.environ.get("BHK_MM_DTYPE", mm_default)
    key = (mode, mm_dtype, nclass, pad, DEBUG)
    if key not in _CACHE:
        _CACHE[key] = _build(mode, mm_dtype, nclass, pad)
    return _CACHE[key]


def kernel(embeddings: np.ndarray, positives_mask: np.ndarray, negatives_mask: np.ndarray) -> np.ndarray:
    from concourse.bass_utils import run_bass_kernel_spmd

    mode, nclass, in_maps = prep(embeddings, positives_mask, negatives_mask)
    nc = get_nc(mode, nclass, _LAST_PREP_INFO["pad"])

    res = run_bass_kernel_spmd(nc, in_maps, core_ids=list(range(NCORES)))
    total = 0.0
    cnt = 0.0
    for r in res.results:
        o = r["out"]
        h = o.shape[1] // 2
        total += float(o[:, 0:h].sum(dtype=np.float64))
        cnt += float(o[:, h:].sum(dtype=np.float64))
    val = np.float32(total / cnt) if cnt > 0 else np.float32(0.0)
    return np.array(val, dtype=np.float32)
